# revision 8
# baseline (speedup 1.0000x reference)
"""Trainium2 Bass kernel for nn_Denoising_ResNet: out = x + conv1x1(box_mean3x3(x)) + b.

Sharding: data-parallel over batch (32 samples -> 4 per core x 8 cores).

I/O precision (rel-err budget 2e-2; compute is bf16 anyway):
  - x is uploaded to device HBM as bf16 (the f32->bf16 rounding is
    identical to the cast-DMA the all-f32 version did on device);
  - y is stored as bf16 and upcast to f32 on the host.
  -> 8.4MB in + 8.4MB out per core (vs 33.6MB all-f32): ~47us DMA floor.

Per-core layout: 2 "stacks" of 2 samples each -> 128 SBUF partitions
(= 2 samples x 64 channels). Each stack's image loads into one bf16 SBUF
tile via quarter DMAs at row boundaries 34/66/98 (chunk ci's halo span
only needs quarters 0..ci). Stack0's quarters are issued upfront (and its
first quarter split in two, with the first 18-row sliver emitted BEFORE
the const loads so the DVE chain starts as early as possible); stack1's
loads are released one-per-chunk from the scalar ring so they don't
steal ramp bandwidth.

Math decomposition (K=3 edge-clipped box mean, then 1x1 conv), balanced
so PE and DVE land at ~47-49us each (both ~saturated; DMA ~40us):
  - W-direction 3-tap sum ws on DVE: two shifted bf16 adds
    (1.92 elem/ns/partition; DVE runs 2-src bf16 adds at 1x regardless of
    alignment); edge cols folded to 1.5*(2-tap) via tiny strided muls on
    GpSimd (small ops are cheap there; bulk elementwise is not).
  - Global /9 of the box mean is folded into the conv weight.
  - H-direction 3-tap is fused into the conv with only TWO PE passes per
    4-row PSUM group using even-aligned pair sums q[j] = ws[2j]+ws[2j+1]
    (one extra DVE add over half the rows):
      even r: hsum[r] = q[r/2]     + ws[r-1]
      odd  r: hsum[r] = q[(r-1)/2] + ws[r+1]
    pass A streams q rows (j0,j0,j0+1,j0+1) via a stride-0 repeat AP;
    pass B streams ws rows l0+(0,3,2,5) via an overlapping-stride AP
    (custom APs stream permuted/repeated rows at full PE rate; zeroed ws
    halo rows give the edge clip at image boundaries).
  - The residual +x is an accumulating identity matmul (kron(I2,I) bf16)
    emitted FIRST (start=True): it only needs xt, so the PE starts the
    moment a quarter lands; the DVE chain stays off the PE critical path.
  - Image-boundary rows get one extra in-group matmul of 0.5x-weight
    against their q row (-> 1.5x conv = the edge-clip row count fix).
  - 8 warmup matmuls run while the first load lands so the PE HAM clock
    gate opens (1.2 -> 2.4 GHz) before real work (warm matmul = 216ns vs
    427 cold for 512 moving cols; bf16 moving is ISA-capped at 512).
  - PSUM: 8-row tiles (2 banks) x 4 bufs; scalar does PSUM->bf16 SBUF
    evac + bias; stores go per 16 rows on the sync ring (per 8 rows for
    the last chunk to cut the drain tail); first/last chunks split their
    DVE chain in halves to shorten ramp/tail.

Measured: ~71us HW exec (baseline all-f32 version: ~107us). Remaining
time: DVE+PE both ~48us busy (saturated, balanced) + ~2.4us launch +
~8us fixed scope-teardown semaphore protocol + ramp/tail slivers.
Attempted and reverted (measured worse on this tile scheduler): column-
interleaved W-pair DVE scheme (78us), GpSimd hsum offload (79us), edge
muls on DVE (95us).
"""
from contextlib import ExitStack

import bass_rust
import numpy as np

import concourse.bass as bass
import concourse.tile as tile
from concourse import bacc, mybir
from concourse.bass_utils import run_bass_kernel_spmd

B, C, H, W = 32, 64, 128, 128
NCORES = 8
PER = B // NCORES  # samples per core
NSTACK = PER // 2  # 2-sample stacks per core
HC = 32  # chunk height (output rows per chunk)
NCHUNK = H // HC
GROUP_ROWS = 4  # rows per matmul accumulation group (512 f32 = 1 bank)
TILE_ROWS = 8  # rows per PSUM tile (2 banks), 2 groups per tile
NTILE = HC // TILE_ROWS
NGRP = TILE_ROWS // GROUP_ROWS

F32 = mybir.dt.float32
BF16 = mybir.dt.bfloat16


def _rows_ap(t, base_row, dims):
    """AP over a [128, R, W] tile starting at base_row, with custom row dims.

    dims: list of (row_stride_in_rows, size) free dims; an inner (1, W) dim is
    appended. Strides may be 0 (repeat) or overlap — used to stream repeated /
    permuted row patterns into the PE without materializing them in SBUF.
    """
    ap = t[:, base_row : base_row + 1, :].copy()
    part = ap.ap.to_list()[0]
    ap.ap = bass_rust.VecI64Pair([part] + [[s * W, n] for s, n in dims] + [[1, W]])
    return ap


def _build_nc() -> bass.Bass:
    nc = bacc.Bacc("TRN2", debug=False)
    x = nc.dram_tensor("x", [PER * C, H, W], BF16, kind="ExternalInput")
    w9t = nc.dram_tensor("w9t", [2 * C, 2 * C], BF16, kind="ExternalInput")
    ident = nc.dram_tensor("ident", [2 * C, 2 * C], BF16, kind="ExternalInput")
    w05t = nc.dram_tensor("w05t", [2 * C, 2 * C], BF16, kind="ExternalInput")
    bias2 = nc.dram_tensor("bias2", [2 * C, 1], F32, kind="ExternalInput")
    y = nc.dram_tensor("y", [PER * C, H, W], BF16, kind="ExternalOutput")
    xap = x.ap()
    yap = y.ap()
    IDENT_FN = mybir.ActivationFunctionType.Identity

    with ExitStack() as ctx:
        tc = ctx.enter_context(tile.TileContext(nc))
        cpool = ctx.enter_context(tc.tile_pool(name="const", bufs=1))
        wt = cpool.tile([128, 128], BF16)
        nc.sync.dma_start(out=wt[:], in_=w9t.ap()[:, :])
        it = cpool.tile([128, 128], BF16)
        nc.sync.dma_start(out=it[:], in_=ident.ap()[:, :])
        w05 = cpool.tile([128, 128], BF16)
        nc.sync.dma_start(out=w05[:], in_=w05t.ap()[:, :])
        bt = cpool.tile([128, 1], F32)
        nc.sync.dma_start(out=bt[:], in_=bias2.ap()[:, :])

        ppool = ctx.enter_context(tc.tile_pool(name="psum", bufs=4, space="PSUM"))

        xpool = ctx.enter_context(tc.tile_pool(name="xin", bufs=2))
        tpool = ctx.enter_context(tc.tile_pool(name="tmp", bufs=4))
        wpool = ctx.enter_context(tc.tile_pool(name="wsum", bufs=4))
        qpool = ctx.enter_context(tc.tile_pool(name="qsum", bufs=4))
        opool = ctx.enter_context(tc.tile_pool(name="out", bufs=4))

        # PE warmup: ~3.4us of dummy matmuls while the first loads land, so the
        # HAM clock gate opens (1.2 -> 2.4 GHz) before the real matmuls start.
        warm = ppool.tile([128, TILE_ROWS, W], F32, tag="ps", name="ps_t")
        wrep = wt[:].copy()
        wrep.ap = bass_rust.VecI64Pair([wrep.ap.to_list()[0], [0, 4], [1, W]])
        for _ in range(6):
            nc.tensor.matmul(warm[:, 0:4, :], wt[:], wrep, start=True, stop=True)

        # quarter boundaries at 34/66/98: chunk ci's halo span
        # [32*ci-1, 32*ci+33) is covered by quarters 0..ci, so each
        # chunk waits only one new quarter-DMA.
        qb = [0, HC + 2, 2 * HC + 2, 3 * HC + 2, H]
        xts = []
        for g in range(NSTACK):
            xt = xpool.tile([128, H, W], BF16, tag="xt", name="xt")
            xts.append(xt)
        # stack0's quarters are issued upfront; stack1's are emitted later in
        # the scalar engine's program order (interleaved with chunk evacs) so
        # stack0's loads get the full DMA bandwidth during the ramp.
        # quarter0 split in two so the first DVE op starts ~1.5us earlier
        nc.scalar.dma_start(out=xts[0][:, 0:18, :], in_=xap[0:128, 0:18, :])
        nc.scalar.dma_start(out=xts[0][:, 18:34, :], in_=xap[0:128, 18:34, :])
        for q in range(1, 4):
            nc.scalar.dma_start(
                out=xts[0][:, qb[q] : qb[q + 1], :],
                in_=xap[0:128, qb[q] : qb[q + 1], :],
            )

        # ps allocations + identity (pass C) emission, callable out-of-band:
        # the NEXT chunk's C passes are interleaved into the CURRENT chunk's
        # tail (right after the evacs that free their PSUM slots) so the PE
        # fills the chunk-boundary dependency waits with ready work.
        PSS = {}

        def alloc_and_c(gg, cc, tps):
            h0c = cc * HC
            pd = PSS.setdefault((gg, cc), {})
            for tp in tps:
                ps_t = ppool.tile([128, TILE_ROWS, W], F32, tag="ps", name="ps_t")
                pd[tp] = ps_t
                t0c = tp * TILE_ROWS
                for hp in range(NGRP):
                    ga, gb = hp * GROUP_ROWS, (hp + 1) * GROUP_ROWS
                    nc.tensor.matmul(
                        ps_t[:, ga:gb, :],
                        it[:],
                        xts[gg][:, h0c + t0c + ga : h0c + t0c + gb, :],
                        start=True,
                        stop=False,
                    )

        alloc_and_c(0, 0, (0, 1, 2, 3))

        for g in range(NSTACK):
            p0 = g * 128
            xt = xts[g]
            for ci in range(NCHUNK):
                h0 = ci * HC
                # chunk rows [h0, h0+HC); halo rows clamped at the image edge
                ra = 0 if ci == 0 else h0 - 1       # first xt row read
                rb = h0 + HC if ci == NCHUNK - 1 else h0 + HC + 1

                # W-direction 3-tap on DVE (bf16). tt/ws row r holds
                # image row h0-1+r; out-of-image halo ws rows are zeroed.
                la = ra - (h0 - 1)  # first valid local row (0 or 1)
                lb = rb - (h0 - 1)  # past-last valid local row
                tt = tpool.tile([128, HC + 2, W], BF16)
                ws = wpool.tile([128, HC + 2, W], BF16)
                if ci == 0:
                    nc.gpsimd.memset(ws[:, 0:1, :], 0.0)
                elif ci == NCHUNK - 1:
                    nc.gpsimd.memset(ws[:, HC + 1 : HC + 2, :], 0.0)
                qt = qpool.tile([128, HC // 2, W], BF16)
                # first chunk (ramp) runs its DVE chain in two row-halves and
                # the last chunk (drain tail) in four quarters so the PE's
                # conv passes start as early as the data allows
                if g == 0 and ci == 0:
                    halves = [(la, 19), (19, lb)]
                elif g == NSTACK - 1 and ci == NCHUNK - 1:
                    halves = [(la, 11), (11, 19), (19, 27), (27, lb)]
                else:
                    halves = [(la, lb)]
                for l1, l2 in halves:
                    x1, x2 = h0 - 1 + l1, h0 - 1 + l2
                    nc.vector.tensor_add(
                        tt[:, l1:l2, 1:W], xt[:, x1:x2, 0 : W - 1], xt[:, x1:x2, 1:W]
                    )
                    nc.vector.tensor_add(
                        ws[:, l1:l2, 1 : W - 1],
                        tt[:, l1:l2, 1 : W - 1],
                        xt[:, x1:x2, 2:W],
                    )
                    # edge-col 1.5x(2-tap) on GpSimd: tiny strided ops are
                    # cheap there (only bulk elementwise is slow), frees DVE.
                    # (scalar would be faster per-op, but the tile scheduler's
                    # cost model then defers the dependent q add behind the
                    # next chunk's W-sums — measured +6.7us)
                    nc.gpsimd.tensor_scalar_mul(ws[:, l1:l2, 0:1], tt[:, l1:l2, 1:2], 1.5)
                    nc.gpsimd.tensor_scalar_mul(
                        ws[:, l1:l2, W - 1 : W], tt[:, l1:l2, W - 1 : W], 1.5
                    )
                    # even-aligned H pair sums on DVE:
                    # q[j] = ws_img[h0+2j] + ws_img[h0+2j+1] (locals 2j+1, 2j+2)
                    j1 = max(0, l1 // 2)
                    j2 = (l2 - 3) // 2
                    nc.vector.tensor_add(
                        qt[:, j1 : j2 + 1, :],
                        ws[:, 2 * j1 + 1 : 2 * j2 + 2 : 2, :],
                        ws[:, 2 * j1 + 2 : 2 * j2 + 3 : 2, :],
                    )

                ot = opool.tile([128, HC, W], BF16)
                pss = PSS.pop((g, ci))
                cg = g * NCHUNK + ci  # global chunk index
                nxt = None if cg + 1 >= NSTACK * NCHUNK else (
                    (cg + 1) // NCHUNK,
                    (cg + 1) % NCHUNK,
                )
                # H-direction 3-tap fused into the conv with only 2 PE
                # passes per 4-row group via the pair sums:
                #   even r: hsum[r] = q[r/2]     + ws[r-1]
                #   odd  r: hsum[r] = q[(r-1)/2] + ws[r+1]
                # pass B streams ws locals l0+(0,3,2,5) [overlapping dims],
                # pass A streams q rows (j0,j0,j0+1,j0+1) [stride-0 repeat].
                for tp in range(NTILE):
                    ps = pss[tp]
                    t0 = tp * TILE_ROWS
                    for hp in range(NGRP):
                        ga, gb = hp * GROUP_ROWS, (hp + 1) * GROUP_ROWS
                        l0 = t0 + ga  # chunk-local first output row of group
                        apB = _rows_ap(ws, l0, [(2, 2), (3, 2)])
                        nc.tensor.matmul(
                            ps[:, ga:gb, :], wt[:], apB, start=False, stop=False
                        )
                        apA = _rows_ap(qt, l0 // 2, [(1, 2), (0, 2)])
                        # image-boundary row: count fix (1.5x conv total); the
                        # clipped 2-row hsum for that row is exactly one q row
                        fix = None
                        if ci == 0 and l0 == 0:
                            fix = 0
                        elif ci == NCHUNK - 1 and l0 == HC - GROUP_ROWS:
                            fix = HC // 2 - 1
                        nc.tensor.matmul(
                            ps[:, ga:gb, :], wt[:], apA, start=False, stop=(fix is None)
                        )
                        if fix is not None:
                            fr = 0 if fix == 0 else TILE_ROWS - 1
                            nc.tensor.matmul(
                                ps[:, fr : fr + 1, :],
                                w05[:],
                                qt[:, fix : fix + 1, :],
                                start=False,
                                stop=True,
                            )
                    nc.scalar.activation(
                        ot[:, t0 : t0 + TILE_ROWS, :],
                        ps[:],
                        IDENT_FN,
                        bias=bt[:],
                    )
                    # store per 2 PSUM tiles (16 rows): finer store pipelining,
                    # shorter drain tail after the last evac
                    last_chunk = g == NSTACK - 1 and ci == NCHUNK - 1
                    if last_chunk:
                        # last chunk: store per tile (8 rows), emitted right
                        # after each evac, to cut release+drain latency after
                        # the final evac
                        nc.sync.dma_start(
                            out=yap[p0 : p0 + 128, h0 + t0 : h0 + t0 + TILE_ROWS, :],
                            in_=ot[:, t0 : t0 + TILE_ROWS, :],
                        )
                    elif tp % 2 == 1:
                        s0 = (tp - 1) * TILE_ROWS
                        nc.sync.dma_start(
                            out=yap[p0 : p0 + 128, h0 + s0 : h0 + s0 + 2 * TILE_ROWS, :],
                            in_=ot[:, s0 : s0 + 2 * TILE_ROWS, :],
                        )
                    if nxt is not None and tp % 2 == 1:
                        # next chunk's C passes for the two slots just freed
                        alloc_and_c(nxt[0], nxt[1], (0, 1) if tp == 1 else (2, 3))
                if g == 0:
                    # release stack1's quarter-ci load now (scalar program
                    # order): stack0's loads got the ramp bandwidth to
                    # themselves, stack1's arrive well before they're needed
                    nc.scalar.dma_start(
                        out=xts[1][:, qb[ci] : qb[ci + 1], :],
                        in_=xap[128:256, qb[ci] : qb[ci + 1], :],
                    )
    nc.compile()
    return nc


_NC = None


def _get_nc() -> bass.Bass:
    global _NC
    if _NC is None:
        _NC = _build_nc()
    return _NC


def _host_inputs(x: np.ndarray, conv_w: np.ndarray, conv_b: np.ndarray):
    import ml_dtypes

    bf = ml_dtypes.bfloat16
    conv_w = np.asarray(conv_w)
    conv_b = np.asarray(conv_b)
    x = np.asarray(x)
    w9t = np.zeros((2 * C, 2 * C), dtype=np.float32)
    wT = (conv_w.astype(np.float32) / 9.0).T
    w9t[0:C, 0:C] = wT
    w9t[C : 2 * C, C : 2 * C] = wT
    ident = np.eye(2 * C, dtype=np.float32).astype(bf)
    w05t = (w9t * 0.5).astype(bf)
    bias2 = np.concatenate([conv_b, conv_b]).reshape(2 * C, 1).astype(np.float32)
    x = np.ascontiguousarray(x, dtype=np.float32).astype(bf)
    in_maps = []
    for i in range(NCORES):
        xi = x[i * PER : (i + 1) * PER].reshape(PER * C, H, W)
        in_maps.append(
            {
                "x": xi,
                "w9t": w9t.astype(bf),
                "ident": ident,
                "w05t": w05t,
                "bias2": bias2,
            }
        )
    return in_maps


def kernel(x: np.ndarray, conv_w: np.ndarray, conv_b: np.ndarray) -> np.ndarray:
    nc = _get_nc()
    in_maps = _host_inputs(x, conv_w, conv_b)
    res = run_bass_kernel_spmd(nc, in_maps, list(range(NCORES)))
    outs = [
        np.asarray(res.results[i]["y"]).astype(np.float32).reshape(PER, C, H, W)
        for i in range(NCORES)
    ]
    return np.concatenate(outs, axis=0)


# revision 9
# speedup vs baseline: 1.0012x; 1.0012x over previous
"""Trainium2 Bass kernel for nn_Denoising_ResNet: out = x + conv1x1(box_mean3x3(x)) + b.

Sharding: data-parallel over batch (32 samples -> 4 per core x 8 cores).

I/O precision (rel-err budget 2e-2; compute is bf16 anyway):
  - x is uploaded to device HBM as bf16 (the f32->bf16 rounding is
    identical to the cast-DMA the all-f32 version did on device);
  - y is stored as bf16 and upcast to f32 on the host.
  -> 8.4MB in + 8.4MB out per core (vs 33.6MB all-f32): ~47us DMA floor.

Per-core layout: 2 "stacks" of 2 samples each -> 128 SBUF partitions
(= 2 samples x 64 channels). Each stack's image loads into one bf16 SBUF
tile via quarter DMAs at row boundaries 34/66/98 (chunk ci's halo span
only needs quarters 0..ci). Stack0's quarters are issued upfront (and its
first quarter split in two, with the first 18-row sliver emitted BEFORE
the const loads so the DVE chain starts as early as possible); stack1's
loads are released one-per-chunk from the scalar ring so they don't
steal ramp bandwidth.

Math decomposition (K=3 edge-clipped box mean, then 1x1 conv), balanced
so PE and DVE land at ~47-49us each (both ~saturated; DMA ~40us):
  - W-direction 3-tap sum ws on DVE: two shifted bf16 adds
    (1.92 elem/ns/partition; DVE runs 2-src bf16 adds at 1x regardless of
    alignment); edge cols folded to 1.5*(2-tap) via tiny strided muls on
    GpSimd (small ops are cheap there; bulk elementwise is not).
  - Global /9 of the box mean is folded into the conv weight.
  - H-direction 3-tap is fused into the conv with only TWO PE passes per
    4-row PSUM group using even-aligned pair sums q[j] = ws[2j]+ws[2j+1]
    (one extra DVE add over half the rows):
      even r: hsum[r] = q[r/2]     + ws[r-1]
      odd  r: hsum[r] = q[(r-1)/2] + ws[r+1]
    pass A streams q rows (j0,j0,j0+1,j0+1) via a stride-0 repeat AP;
    pass B streams ws rows l0+(0,3,2,5) via an overlapping-stride AP
    (custom APs stream permuted/repeated rows at full PE rate; zeroed ws
    halo rows give the edge clip at image boundaries).
  - The residual +x is an accumulating identity matmul (kron(I2,I) bf16)
    emitted FIRST (start=True): it only needs xt, so the PE starts the
    moment a quarter lands; the DVE chain stays off the PE critical path.
  - Image-boundary rows get one extra in-group matmul of 0.5x-weight
    against their q row (-> 1.5x conv = the edge-clip row count fix).
  - 8 warmup matmuls run while the first load lands so the PE HAM clock
    gate opens (1.2 -> 2.4 GHz) before real work (warm matmul = 216ns vs
    427 cold for 512 moving cols; bf16 moving is ISA-capped at 512).
  - PSUM: 8-row tiles (2 banks) x 4 bufs; scalar does PSUM->bf16 SBUF
    evac + bias; stores go per 16 rows on the sync ring (per 8 rows for
    the last chunk to cut the drain tail); first/last chunks split their
    DVE chain in halves to shorten ramp/tail.

Measured: ~71us HW exec (baseline all-f32 version: ~107us). Remaining
time: DVE+PE both ~48us busy (saturated, balanced) + ~2.4us launch +
~8us fixed scope-teardown semaphore protocol + ramp/tail slivers.
Attempted and reverted (measured worse on this tile scheduler): column-
interleaved W-pair DVE scheme (78us), GpSimd hsum offload (79us), edge
muls on DVE (95us).
"""
from contextlib import ExitStack

import bass_rust
import numpy as np

import concourse.bass as bass
import concourse.tile as tile
from concourse import bacc, mybir
from concourse.bass_utils import run_bass_kernel_spmd

B, C, H, W = 32, 64, 128, 128
NCORES = 8
PER = B // NCORES  # samples per core
NSTACK = PER // 2  # 2-sample stacks per core
HC = 32  # chunk height (output rows per chunk)
NCHUNK = H // HC
GROUP_ROWS = 4  # rows per matmul accumulation group (512 f32 = 1 bank)
TILE_ROWS = 8  # rows per PSUM tile (2 banks), 2 groups per tile
NTILE = HC // TILE_ROWS
NGRP = TILE_ROWS // GROUP_ROWS

F32 = mybir.dt.float32
BF16 = mybir.dt.bfloat16


def _rows_ap(t, base_row, dims):
    """AP over a [128, R, W] tile starting at base_row, with custom row dims.

    dims: list of (row_stride_in_rows, size) free dims; an inner (1, W) dim is
    appended. Strides may be 0 (repeat) or overlap — used to stream repeated /
    permuted row patterns into the PE without materializing them in SBUF.
    """
    ap = t[:, base_row : base_row + 1, :].copy()
    part = ap.ap.to_list()[0]
    ap.ap = bass_rust.VecI64Pair([part] + [[s * W, n] for s, n in dims] + [[1, W]])
    return ap


def _build_nc() -> bass.Bass:
    nc = bacc.Bacc("TRN2", debug=False)
    x = nc.dram_tensor("x", [PER * C, H, W], BF16, kind="ExternalInput")
    w9t = nc.dram_tensor("w9t", [2 * C, 2 * C], BF16, kind="ExternalInput")
    ident = nc.dram_tensor("ident", [2 * C, 2 * C], BF16, kind="ExternalInput")
    w05t = nc.dram_tensor("w05t", [2 * C, 2 * C], BF16, kind="ExternalInput")
    bias2 = nc.dram_tensor("bias2", [2 * C, 1], F32, kind="ExternalInput")
    y = nc.dram_tensor("y", [PER * C, H, W], BF16, kind="ExternalOutput")
    xap = x.ap()
    yap = y.ap()
    IDENT_FN = mybir.ActivationFunctionType.Identity

    with ExitStack() as ctx:
        tc = ctx.enter_context(tile.TileContext(nc))
        cpool = ctx.enter_context(tc.tile_pool(name="const", bufs=1))
        wt = cpool.tile([128, 128], BF16)
        nc.sync.dma_start(out=wt[:], in_=w9t.ap()[:, :])
        it = cpool.tile([128, 128], BF16)
        nc.sync.dma_start(out=it[:], in_=ident.ap()[:, :])
        w05 = cpool.tile([128, 128], BF16)
        nc.sync.dma_start(out=w05[:], in_=w05t.ap()[:, :])
        bt = cpool.tile([128, 1], F32)
        nc.sync.dma_start(out=bt[:], in_=bias2.ap()[:, :])

        ppool = ctx.enter_context(tc.tile_pool(name="psum", bufs=4, space="PSUM"))

        xpool = ctx.enter_context(tc.tile_pool(name="xin", bufs=2))
        tpool = ctx.enter_context(tc.tile_pool(name="tmp", bufs=4))
        wpool = ctx.enter_context(tc.tile_pool(name="wsum", bufs=4))
        qpool = ctx.enter_context(tc.tile_pool(name="qsum", bufs=4))
        opool = ctx.enter_context(tc.tile_pool(name="out", bufs=4))

        # PE warmup: ~3.4us of dummy matmuls while the first loads land, so the
        # HAM clock gate opens (1.2 -> 2.4 GHz) before the real matmuls start.
        warm = ppool.tile([128, TILE_ROWS, W], F32, tag="ps", name="ps_t")
        wrep = wt[:].copy()
        wrep.ap = bass_rust.VecI64Pair([wrep.ap.to_list()[0], [0, 4], [1, W]])
        for _ in range(6):
            nc.tensor.matmul(warm[:, 0:4, :], wt[:], wrep, start=True, stop=True)

        # quarter boundaries at 34/66/98: chunk ci's halo span
        # [32*ci-1, 32*ci+33) is covered by quarters 0..ci, so each
        # chunk waits only one new quarter-DMA.
        qb = [0, HC + 2, 2 * HC + 2, 3 * HC + 2, H]
        xts = []
        for g in range(NSTACK):
            xt = xpool.tile([128, H, W], BF16, tag="xt", name="xt")
            xts.append(xt)
        # stack0's quarters are issued upfront; stack1's are emitted later in
        # the scalar engine's program order (interleaved with chunk evacs) so
        # stack0's loads get the full DMA bandwidth during the ramp.
        # quarter0 split in two so the first DVE op starts ~1.5us earlier
        nc.scalar.dma_start(out=xts[0][:, 0:18, :], in_=xap[0:128, 0:18, :])
        nc.scalar.dma_start(out=xts[0][:, 18:34, :], in_=xap[0:128, 18:34, :])
        for q in range(1, 4):
            nc.scalar.dma_start(
                out=xts[0][:, qb[q] : qb[q + 1], :],
                in_=xap[0:128, qb[q] : qb[q + 1], :],
            )

        # ps allocations + identity (pass C) emission, callable out-of-band:
        # the NEXT chunk's C passes are interleaved into the CURRENT chunk's
        # tail (right after the evacs that free their PSUM slots) so the PE
        # fills the chunk-boundary dependency waits with ready work.
        PSS = {}

        def alloc_and_c(gg, cc, tps):
            h0c = cc * HC
            pd = PSS.setdefault((gg, cc), {})
            for tp in tps:
                ps_t = ppool.tile([128, TILE_ROWS, W], F32, tag="ps", name="ps_t")
                pd[tp] = ps_t
                t0c = tp * TILE_ROWS
                for hp in range(NGRP):
                    ga, gb = hp * GROUP_ROWS, (hp + 1) * GROUP_ROWS
                    nc.tensor.matmul(
                        ps_t[:, ga:gb, :],
                        it[:],
                        xts[gg][:, h0c + t0c + ga : h0c + t0c + gb, :],
                        start=True,
                        stop=False,
                    )

        alloc_and_c(0, 0, (0, 1, 2, 3))

        for g in range(NSTACK):
            p0 = g * 128
            xt = xts[g]
            for ci in range(NCHUNK):
                h0 = ci * HC
                # chunk rows [h0, h0+HC); halo rows clamped at the image edge
                ra = 0 if ci == 0 else h0 - 1       # first xt row read
                rb = h0 + HC if ci == NCHUNK - 1 else h0 + HC + 1

                # W-direction 3-tap on DVE (bf16). tt/ws row r holds
                # image row h0-1+r; out-of-image halo ws rows are zeroed.
                la = ra - (h0 - 1)  # first valid local row (0 or 1)
                lb = rb - (h0 - 1)  # past-last valid local row
                tt = tpool.tile([128, HC + 2, W], BF16)
                ws = wpool.tile([128, HC + 2, W], BF16)
                if ci == 0:
                    nc.gpsimd.memset(ws[:, 0:1, :], 0.0)
                elif ci == NCHUNK - 1:
                    nc.gpsimd.memset(ws[:, HC + 1 : HC + 2, :], 0.0)
                qt = qpool.tile([128, HC // 2, W], BF16)
                # first chunk (ramp) and last chunk (drain tail) run their DVE
                # chains in two row-halves so the PE's conv passes start earlier
                # (four-quarter split on the last chunk measured +0.5us)
                split = (g == 0 and ci == 0) or (
                    g == NSTACK - 1 and ci == NCHUNK - 1
                )
                halves = [(la, 19), (19, lb)] if split else [(la, lb)]
                for l1, l2 in halves:
                    x1, x2 = h0 - 1 + l1, h0 - 1 + l2
                    nc.vector.tensor_add(
                        tt[:, l1:l2, 1:W], xt[:, x1:x2, 0 : W - 1], xt[:, x1:x2, 1:W]
                    )
                    nc.vector.tensor_add(
                        ws[:, l1:l2, 1 : W - 1],
                        tt[:, l1:l2, 1 : W - 1],
                        xt[:, x1:x2, 2:W],
                    )
                    # edge-col 1.5x(2-tap) on GpSimd: tiny strided ops are
                    # cheap there (only bulk elementwise is slow), frees DVE.
                    # (scalar would be faster per-op, but the tile scheduler's
                    # cost model then defers the dependent q add behind the
                    # next chunk's W-sums — measured +6.7us)
                    nc.gpsimd.tensor_scalar_mul(ws[:, l1:l2, 0:1], tt[:, l1:l2, 1:2], 1.5)
                    nc.gpsimd.tensor_scalar_mul(
                        ws[:, l1:l2, W - 1 : W], tt[:, l1:l2, W - 1 : W], 1.5
                    )
                    # even-aligned H pair sums on DVE:
                    # q[j] = ws_img[h0+2j] + ws_img[h0+2j+1] (locals 2j+1, 2j+2)
                    j1 = max(0, l1 // 2)
                    j2 = (l2 - 3) // 2
                    nc.vector.tensor_add(
                        qt[:, j1 : j2 + 1, :],
                        ws[:, 2 * j1 + 1 : 2 * j2 + 2 : 2, :],
                        ws[:, 2 * j1 + 2 : 2 * j2 + 3 : 2, :],
                    )

                ot = opool.tile([128, HC, W], BF16)
                pss = PSS.pop((g, ci))
                cg = g * NCHUNK + ci  # global chunk index
                nxt = None if cg + 1 >= NSTACK * NCHUNK else (
                    (cg + 1) // NCHUNK,
                    (cg + 1) % NCHUNK,
                )
                # H-direction 3-tap fused into the conv with only 2 PE
                # passes per 4-row group via the pair sums:
                #   even r: hsum[r] = q[r/2]     + ws[r-1]
                #   odd  r: hsum[r] = q[(r-1)/2] + ws[r+1]
                # pass B streams ws locals l0+(0,3,2,5) [overlapping dims],
                # pass A streams q rows (j0,j0,j0+1,j0+1) [stride-0 repeat].
                for tp in range(NTILE):
                    ps = pss[tp]
                    t0 = tp * TILE_ROWS
                    for hp in range(NGRP):
                        ga, gb = hp * GROUP_ROWS, (hp + 1) * GROUP_ROWS
                        l0 = t0 + ga  # chunk-local first output row of group
                        apB = _rows_ap(ws, l0, [(2, 2), (3, 2)])
                        nc.tensor.matmul(
                            ps[:, ga:gb, :], wt[:], apB, start=False, stop=False
                        )
                        apA = _rows_ap(qt, l0 // 2, [(1, 2), (0, 2)])
                        # image-boundary row: count fix (1.5x conv total); the
                        # clipped 2-row hsum for that row is exactly one q row
                        fix = None
                        if ci == 0 and l0 == 0:
                            fix = 0
                        elif ci == NCHUNK - 1 and l0 == HC - GROUP_ROWS:
                            fix = HC // 2 - 1
                        nc.tensor.matmul(
                            ps[:, ga:gb, :], wt[:], apA, start=False, stop=(fix is None)
                        )
                        if fix is not None:
                            fr = 0 if fix == 0 else TILE_ROWS - 1
                            nc.tensor.matmul(
                                ps[:, fr : fr + 1, :],
                                w05[:],
                                qt[:, fix : fix + 1, :],
                                start=False,
                                stop=True,
                            )
                    nc.scalar.activation(
                        ot[:, t0 : t0 + TILE_ROWS, :],
                        ps[:],
                        IDENT_FN,
                        bias=bt[:],
                    )
                    # store per 2 PSUM tiles (16 rows): finer store pipelining,
                    # shorter drain tail after the last evac
                    last_chunk = g == NSTACK - 1 and ci == NCHUNK - 1
                    if last_chunk:
                        # last chunk: store per tile (8 rows), emitted right
                        # after each evac, to cut release+drain latency after
                        # the final evac
                        nc.sync.dma_start(
                            out=yap[p0 : p0 + 128, h0 + t0 : h0 + t0 + TILE_ROWS, :],
                            in_=ot[:, t0 : t0 + TILE_ROWS, :],
                        )
                    elif tp % 2 == 1:
                        s0 = (tp - 1) * TILE_ROWS
                        nc.sync.dma_start(
                            out=yap[p0 : p0 + 128, h0 + s0 : h0 + s0 + 2 * TILE_ROWS, :],
                            in_=ot[:, s0 : s0 + 2 * TILE_ROWS, :],
                        )
                    if nxt is not None and tp % 2 == 1:
                        # next chunk's C passes for the two slots just freed
                        alloc_and_c(nxt[0], nxt[1], (0, 1) if tp == 1 else (2, 3))
                if g == 0:
                    # release stack1's quarter-ci load now (scalar program
                    # order): stack0's loads got the ramp bandwidth to
                    # themselves, stack1's arrive well before they're needed
                    nc.scalar.dma_start(
                        out=xts[1][:, qb[ci] : qb[ci + 1], :],
                        in_=xap[128:256, qb[ci] : qb[ci + 1], :],
                    )
    nc.compile()
    return nc


_NC = None


def _get_nc() -> bass.Bass:
    global _NC
    if _NC is None:
        _NC = _build_nc()
    return _NC


def _host_inputs(x: np.ndarray, conv_w: np.ndarray, conv_b: np.ndarray):
    import ml_dtypes

    bf = ml_dtypes.bfloat16
    conv_w = np.asarray(conv_w)
    conv_b = np.asarray(conv_b)
    x = np.asarray(x)
    w9t = np.zeros((2 * C, 2 * C), dtype=np.float32)
    wT = (conv_w.astype(np.float32) / 9.0).T
    w9t[0:C, 0:C] = wT
    w9t[C : 2 * C, C : 2 * C] = wT
    ident = np.eye(2 * C, dtype=np.float32).astype(bf)
    w05t = (w9t * 0.5).astype(bf)
    bias2 = np.concatenate([conv_b, conv_b]).reshape(2 * C, 1).astype(np.float32)
    x = np.ascontiguousarray(x, dtype=np.float32).astype(bf)
    in_maps = []
    for i in range(NCORES):
        xi = x[i * PER : (i + 1) * PER].reshape(PER * C, H, W)
        in_maps.append(
            {
                "x": xi,
                "w9t": w9t.astype(bf),
                "ident": ident,
                "w05t": w05t,
                "bias2": bias2,
            }
        )
    return in_maps


def kernel(x: np.ndarray, conv_w: np.ndarray, conv_b: np.ndarray) -> np.ndarray:
    nc = _get_nc()
    in_maps = _host_inputs(x, conv_w, conv_b)
    res = run_bass_kernel_spmd(nc, in_maps, list(range(NCORES)))
    outs = [
        np.asarray(res.results[i]["y"]).astype(np.float32).reshape(PER, C, H, W)
        for i in range(NCORES)
    ]
    return np.concatenate(outs, axis=0)
